# revision 14
# baseline (speedup 1.0000x reference)
"""TAGConv x2 + PReLU + global_add_pool, instruction-minimal for trn2 x8.

Design (per core, feature-major [64 part, nodes]):
 - G table in DRAM: [NPAD, 128] bf16 rows (64 feats + 64 zero pad, 256B).
 - Hop: dma_gather(transpose=True) pulls edge messages as columns
   (features on partitions 0-63), in octet order (8 same-dst edges per
   octet, zero-padded via a guaranteed-zero position). gpsimd scatter_add
   (d=8) accumulates octets into tab[64, DH+1, 8]; each call has UNIQUE
   dst indices (one octet per dst per rank-call) so the Q7 RMW races on
   duplicate indices never occur. Fold 8 slots (3 strided adds using msgT
   as scratch) then scale by dinv -> hT_k.
 - G exchange: gT=hT_k*dinv -> dma_start_transpose -> node-major gnm ->
   DMA into AG_in rows (upper lanes pre-zeroed) -> AllGather -> G_shared.
 - W phase: 13 psum chunks x 4 matmuls (lhsT=W[k] 64x64) + Prelu act
   (bias as per-partition AP) writing next-layer h (bf16).
 - Pool: dma_start_transpose h1 -> [128,49,64]; 49 accumulating matmuls
   with host-built one-hot Bnm -> PT[64 feat, 128 graph] psum; 1 matmul
   with Wout -> z[1,128]; AllReduce; +bout.
Node id -> position permutation swaps 25087 <-> 50100 so position 25087
(lo half) is a guaranteed-zero row for gather padding (dinv=0 there).
"""
import os
import numpy as np
import ml_dtypes

_V2T = os.environ.get("V2T", "full")
GSUB = int(os.environ.get("GSUB", "12288"))  # max edges per dma_gather

N, NPAD, NC = 50000, 50176, 8
NPC = NPAD // NC            # 6272
DH = NPC // 2               # 3136
K = 3
NGR = 128
HALF = NPAD // 2            # 25088
SWAP_A, SWAP_B = 25087, 50100
CH_OCT = 2048               # octets per gather chunk (x16); CH_OCT*8 >= 5*DH
CH_E = CH_OCT * 8           # 22016 edge columns in msgT
NBLK = NPC // 128           # 49


def _wrap16(idx):
    n = len(idx)
    w = np.zeros((16, (n + 15) // 16), np.int16)
    jj = np.arange(n)
    w[jj % 16, jj // 16] = idx
    return np.tile(w, (8, 1))[:128]


def _host_prep(inputs):
    x = np.asarray(inputs["x"], np.float32)
    edge_index = np.asarray(inputs["edge_index"], np.int64)
    batch_ids = np.asarray(inputs["batch_ids"], np.int64)

    src, dst = edge_index[0], edge_index[1]
    deg = np.bincount(dst, minlength=N).astype(np.float64)
    dinv_n = np.zeros(N, np.float32)
    nz = deg > 0
    dinv_n[nz] = (1.0 / np.sqrt(deg[nz])).astype(np.float32)

    # node -> position permutation
    nodes = np.arange(N)
    pn = np.where(nodes == SWAP_A, SWAP_B, nodes)  # SWAP_B>=N so no clash
    dinv_p = np.zeros(NPAD, np.float32)
    x_p = np.zeros((NPAD, x.shape[1]), np.float32)
    batch_p = np.full(NPAD, NGR, np.int64)
    dinv_p[pn] = dinv_n
    x_p[pn] = x
    batch_p[pn] = batch_ids

    ps = np.where(src == SWAP_A, SWAP_B, src)
    pd = np.where(dst == SWAP_A, SWAP_B, dst)

    # table-row permutation within each shard: local n=c*128+p -> row p*49+c
    # (makes the exchange relay DMA per-partition contiguous)
    nl = np.arange(NPC)
    sigma = (nl % 128) * NBLK + nl // 128
    rs_ = (ps // NPC) * NPC + sigma[ps % NPC]  # src table rows

    # per core, per (dh, sh): octet arrays + per-rank scatter idx
    # core_data[r] = list over (dh, sh) of (list_of_rank_octets, list_of_rank_sidx)
    core_data = []
    for r in range(NC):
        m = (pd >= r * NPC) & (pd < (r + 1) * NPC)
        eps, epd = ps[m], pd[m] % NPC
        ers = rs_[m]
        segs = []
        for dh in range(2):
            for sh in range(2):
                mm = ((epd >= dh * DH) & (epd < (dh + 1) * DH)
                      & (eps >= sh * HALF) & (eps < (sh + 1) * HALF))
                s_ = ers[mm] - sh * HALF
                d_ = epd[mm] - dh * DH
                zr = np.int16(25087)  # zero position, relative (both halves)
                order = np.argsort(d_, kind="stable")
                s_, d_ = s_[order], d_[order]
                cnt = np.bincount(d_, minlength=DH)
                dstart = np.r_[0, np.cumsum(cnt)[:-1]]
                rank_oct, rank_sidx = [], []
                rk = 0
                while True:
                    sel = np.nonzero(cnt > 8 * rk)[0]
                    if len(sel) == 0:
                        break
                    octs = np.full((len(sel), 8), zr, np.int16)
                    for slot in range(8):
                        has = cnt[sel] > 8 * rk + slot
                        octs[has, slot] = s_[dstart[sel[has]] + 8 * rk + slot]
                    rank_oct.append(octs)
                    rank_sidx.append(sel.astype(np.int16))
                    rk += 1
                segs.append((rank_oct, rank_sidx))
        core_data.append(segs)

    # global schedule: per (seg, rank): n_oct = max over cores, rounded x16
    sched = []  # list of (seg_id, rank, n_oct)
    for seg_id in range(4):
        rmax = max(len(core_data[r][seg_id][0]) for r in range(NC))
        for rk in range(rmax):
            n = max((len(core_data[r][seg_id][0][rk])
                     if rk < len(core_data[r][seg_id][0]) else 0)
                    for r in range(NC))
            n = ((n + 15) // 16) * 16
            sched.append((seg_id, rk, n))

    # chunks: greedy within seg, splitting calls at x16 boundaries
    # chunk = (sh, [(call_id, off_oct, n_oct_sub)...])
    chunks = []
    cur = None
    cur_fill = 0
    for cid, (seg_id, rk, n) in enumerate(sched):
        sh = seg_id % 2
        off = 0
        while off < n:
            if cur is None or cur[0] != (seg_id // 2, sh) or cur_fill >= CH_OCT:
                cur = ((seg_id // 2, sh), [])
                chunks.append(cur)
                cur_fill = 0
            take = min(CH_OCT - cur_fill, n - off)
            cur[1].append((cid, off, take))
            cur_fill += take
            off += take
    return (x_p, dinv_p, batch_p, core_data, sched, chunks)


def _build(inputs, repeat=1):
    import concourse.bacc as bacc
    import concourse.mybir as mybir
    import concourse.tile as tile
    from concourse.library_config import mlp
    from concourse.bass import _add_dep_helper

    f32 = mybir.dt.float32
    bf16 = mybir.dt.bfloat16
    i16 = mybir.dt.int16

    W0 = np.asarray(inputs["W0"], np.float32)
    b0 = np.asarray(inputs["b0"], np.float32)
    W1 = np.asarray(inputs["W1"], np.float32)
    b1 = np.asarray(inputs["b1"], np.float32)
    alphas = [float(np.asarray(inputs["alpha0"]).reshape(-1)[0]),
              float(np.asarray(inputs["alpha1"]).reshape(-1)[0])]
    Wout = np.asarray(inputs["Wout"], np.float32)
    bout = float(np.asarray(inputs["bout"]).reshape(-1)[0])

    x_p, dinv_p, batch_p, core_data, sched, chunks = _host_prep(inputs)

    g0_rows = np.zeros((NPAD, 128), np.float32)
    g0_rows[:, 0:64] = x_p * dinv_p[:, None]
    nl = np.arange(NPC)
    sigma = (nl % 128) * NBLK + nl // 128
    for r in range(NC):
        blk = g0_rows[r * NPC:(r + 1) * NPC].copy()
        g0_rows[r * NPC + sigma] = blk

    # per-core blobs following sched
    per_core = []
    for r in range(NC):
        g_parts, s_parts = [], []
        for seg_id, rk, n in sched:
            ro, rs = core_data[r][seg_id]
            if rk < len(ro):
                octs, sidx = ro[rk], rs[rk]
            else:
                octs = np.zeros((0, 8), np.int16)
                sidx = np.zeros(0, np.int16)
            pad = n - len(octs)
            octs = np.vstack([octs, np.full((pad, 8), 25087, np.int16)])
            sidx = np.r_[sidx, np.full(pad, DH, np.int16)]
            g_parts.append(octs.reshape(-1))
            s_parts.append(sidx)
        gblob = np.concatenate(g_parts)
        sblob = np.concatenate(s_parts)
        sl = slice(r * NPC, (r + 1) * NPC)
        xT = np.ascontiguousarray(x_p[sl].T)
        dinvT = np.tile(dinv_p[sl][None, :], (64, 1))
        bp = batch_p[sl].reshape(NBLK, 128)
        Bnm = np.zeros((128, NBLK, 128), np.float32)
        for b in range(NBLK):
            valid = bp[b] < NGR
            Bnm[np.arange(128)[valid], b, bp[b][valid]] = 1.0
        m = dict(
            g0=g0_rows.astype(ml_dtypes.bfloat16).view(np.int16),
            gidx=_wrap16(gblob),
            sidx=_wrap16(sblob),
            xT=xT.astype(ml_dtypes.bfloat16).view(np.int16),
            dinvT=dinvT.astype(ml_dtypes.bfloat16).view(np.int16),
            Wsb=np.ascontiguousarray(
                np.stack([W0, W1]).transpose(2, 0, 1, 3).reshape(64, 512)
            ).astype(ml_dtypes.bfloat16).view(np.int16),
            Woutb=Wout.astype(ml_dtypes.bfloat16).view(np.int16),
            bb=np.stack([b0, b1], 1).astype(np.float32),
            Bnm=np.ascontiguousarray(Bnm.reshape(128, NBLK * 128)
                                     ).astype(ml_dtypes.bfloat16).view(np.int16),
        )
        per_core.append(m)

    GW = per_core[0]["gidx"].shape[1]
    SW = per_core[0]["sidx"].shape[1]

    nc = bacc.Bacc("TRN2", target_bir_lowering=False, debug=False,
                   num_devices=NC, dynamic_dma_scratch_size=32768)

    def ein(name, shape, dtype=f32):
        return nc.dram_tensor(name, shape, dtype, kind="ExternalInput")

    g0_e = ein("g0", [NPAD, 128], i16)
    gidx_e = ein("gidx", [128, GW], i16)
    sidx_e = ein("sidx", [128, SW], i16)
    xT_e = ein("xT", [64, NPC], i16)
    dinvT_e = ein("dinvT", [64, NPC], i16)
    Wsb_e = ein("Wsb", [64, 8 * 64], i16)
    Wout_e = ein("Woutb", [64, 1], i16)
    bb_e = ein("bb", [64, 2])
    Bnm_e = ein("Bnm", [128, NBLK * 128], i16)
    out_e = nc.dram_tensor("out", [NGR, 1], f32, kind="ExternalOutput")

    G_shared = nc.dram_tensor("G_shared", [NPAD // 8, 512], f32, addr_space="Shared")
    AG_in = nc.dram_tensor("AG_in", [NPC // 8, 512], f32)
    ar_in = nc.dram_tensor("ar_in", [1, NGR], f32)
    ar_out = nc.dram_tensor("ar_out", [1, NGR], f32, addr_space="Shared")
    RG = [list(range(NC))]

    # per-call scatter idx offsets (in octets)
    soffs = np.r_[0, np.cumsum([n for _, _, n in sched])]

    with tile.TileContext(nc) as tc:
        with (
            tc.tile_pool(name="c", bufs=1) as cpool,
            tc.tile_pool(name="w", bufs=3) as wp,
            tc.tile_pool(name="pw", bufs=4, space="PSUM") as pw,
            tc.tile_pool(name="pp", bufs=1, space="PSUM") as pp,
        ):
            lib_i = nc.gpsimd.load_library(mlp)
            regs = {}

            def reg(n):
                if n not in regs:
                    regs[n] = nc.gpsimd.to_reg(n)
                return regs[n]

            msgT = nc.alloc_sbuf_tensor("msgT", [128, CH_E], bf16)
            tab = nc.alloc_sbuf_tensor("tab", [64, DH + 1, 8], bf16)
            hT = [nc.alloc_sbuf_tensor(f"hT{k}", [64, NPC], bf16)
                  for k in range(K + 1)]
            gT = nc.alloc_sbuf_tensor("gT", [64, NPC], bf16)
            gnm = nc.alloc_sbuf_tensor("gnm", [128, NBLK, 128], bf16)
            dinvT = nc.alloc_sbuf_tensor("dinvT_sb", [64, NPC], bf16)
            gidx = nc.alloc_sbuf_tensor("gidx_sb", [128, GW], i16)
            sidx = nc.alloc_sbuf_tensor("sidx_sb", [128, SW], i16)
            Wsb = cpool.tile([64, 8, 64], bf16)
            Woutb = cpool.tile([64, 1], bf16)
            bb = cpool.tile([64, 2], f32)
            PTsb = cpool.tile([64, NGR], bf16)

            nc.sync.dma_start(out=hT[0].ap(), in_=xT_e.ap().bitcast(bf16))
            nc.sync.dma_start(out=dinvT.ap(), in_=dinvT_e.ap().bitcast(bf16))
            nc.sync.dma_start(out=gidx.ap(), in_=gidx_e[:])
            nc.sync.dma_start(out=sidx.ap(), in_=sidx_e[:])
            nc.sync.dma_start(out=Wsb[:].rearrange("p a b -> p (a b)"),
                              in_=Wsb_e.ap().bitcast(bf16))
            nc.sync.dma_start(out=Woutb[:], in_=Wout_e.ap().bitcast(bf16))
            nc.sync.dma_start(out=bb[:], in_=bb_e[:])
            nc.vector.memset(gnm.ap().rearrange("p a b -> p (a b)"), 0.0)

            def do_hop(kk, first, gather_en=True, scatter_en=True):
                src_tab = (g0_e.ap().bitcast(bf16) if first
                           else G_shared.ap().bitcast(bf16)
                           .rearrange("a b -> (a b)")
                           .rearrange("(n f) -> n f", f=128))
                # dh groups: chunks are ordered dh0 then dh1
                cur_dh = -1
                goff = 0  # edge offset into gidx blob
                for (dh, sh), parts in chunks:
                    if dh != cur_dh:
                        if cur_dh >= 0:
                            fold(cur_dh, kk)
                        nc.vector.memset(tab.ap().rearrange("p a b -> p (a b)"), 0.0)
                        cur_dh = dh
                    ne = sum(t * 8 for _, _, t in parts)
                    tab_in = (src_tab[0:HALF, :] if sh == 0
                              else src_tab[HALF:NPAD, :])
                    if gather_en:
                        gs = GSUB if GSUB else ne
                        for sub in range(0, ne, gs):
                            nsub = min(gs, ne - sub)
                            gi = nc.gpsimd.dma_gather(
                                msgT.ap()[:, sub : sub + nsub]
                                    .rearrange("p (one n) -> p one n", one=1),
                                tab_in,
                                gidx.ap()[:, (goff + sub) // 16
                                          : (goff + sub + nsub) // 16],
                                nsub, reg(nsub), 128,
                                transpose=True, single_packet=False,
                            )
                            _add_dep_helper(gi.ins, lib_i.ins, True, "lib first")
                    goff += ne
                    co = 0  # octet offset within chunk
                    for cid, off, t in parts:
                        if not scatter_en:
                            continue
                        so = soffs[cid] + off
                        sa = nc.gpsimd.scatter_add(
                            tab.ap(),
                            sidx.ap()[0:64, so // 16 : (so + t) // 16],
                            msgT.ap()[0:64, co * 8 : (co + t) * 8]
                                .rearrange("p (n d) -> p n d", d=8),
                            64, DH + 1, 8, t,
                        )
                        _add_dep_helper(sa.ins, lib_i.ins, True, "lib first")
                        co += t
                fold(cur_dh, kk)

            def fold(dh, kk):
                """tab[64, DH, 8] -> hT[kk][:, dh*DH:] = sum(slots)*dinv."""
                tv = tab.ap()[:, 0:DH, :]
                tA = msgT.ap()[0:64, 0 : DH * 4].rearrange(
                    "p (n d) -> p n d", d=4)
                tB = gT.ap()[:, 0 : DH * 2].rearrange(
                    "p (n d) -> p n d", d=2)
                tC = msgT.ap()[0:64, DH * 4 : DH * 5]
                nc.vector.tensor_tensor(out=tA, in0=tv[:, :, 0:4],
                                        in1=tv[:, :, 4:8],
                                        op=mybir.AluOpType.add)
                nc.vector.tensor_tensor(out=tB, in0=tA[:, :, 0:2],
                                        in1=tA[:, :, 2:4],
                                        op=mybir.AluOpType.add)
                nc.vector.tensor_tensor(out=tC.rearrange("p (n d) -> p n d", d=1),
                                        in0=tB[:, :, 0:1], in1=tB[:, :, 1:2],
                                        op=mybir.AluOpType.add)
                cols = slice(dh * DH, (dh + 1) * DH)
                nc.vector.tensor_tensor(out=hT[kk].ap()[:, cols], in0=tC,
                                        in1=dinvT.ap()[:, cols],
                                        op=mybir.AluOpType.mult)

            def exchange(src):
                """src [64, NPC] bf16 = g values -> AllGather into G_shared."""
                nc.sync.dma_start_transpose(gnm.ap()[:, :, 0:64], src)
                nc.sync.dma_start(
                    out=AG_in.ap().bitcast(bf16).rearrange("a b -> (a b)")
                        .rearrange("(p c f) -> p c f", p=128, f=128),
                    in_=gnm.ap(),
                )
                nc.gpsimd.collective_compute(
                    "AllGather", mybir.AluOpType.bypass,
                    replica_groups=RG, ins=[AG_in[:]], outs=[G_shared[:]],
                )

            def wphase(layer):
                for c in range(13):
                    c0 = c * 512
                    cw = min(512, NPC - c0)
                    ps = pw.tile([64, 512], f32)
                    for k in range(K + 1):
                        nc.tensor.matmul(
                            ps[:, 0:cw], lhsT=Wsb[:, layer * 4 + k, :],
                            rhs=hT[k].ap()[:, c0 : c0 + cw],
                            start=(k == 0), stop=(k == K),
                        )
                    nc.scalar.activation(
                        hT[0].ap()[:, c0 : c0 + cw], ps[:, 0:cw],
                        mybir.ActivationFunctionType.Prelu,
                        bias=bb[:, layer : layer + 1], scale=1.0,
                        alpha=alphas[layer],
                    )

            def dbg_out(src_bf16_col):
                d = wp.tile([64, 1], f32, tag="dbg")
                nc.vector.tensor_copy(out=d[:], in_=src_bf16_col)
                nc.sync.dma_start(out=out_e[0:64, :], in_=d[:])

            if _V2T == "hops8":
                for _ in range(8):
                    do_hop(1, True)
                dbg_out(hT[1].ap()[:, 0:1])
            elif _V2T == "ag8":
                for _ in range(8):
                    exchange(gT.ap()[:, :])
                dbg_out(hT[1].ap()[:, 0:1])
            elif _V2T == "wp8":
                for _ in range(8):
                    wphase(0)
                dbg_out(hT[0].ap()[:, 0:1])
            elif _V2T == "hop":
                do_hop(1, True)
                dbg_out(hT[1].ap()[:, 0:1])
            elif _V2T == "gonly":
                do_hop(1, True, scatter_en=False)
                dbg_out(hT[1].ap()[:, 0:1])
            elif _V2T == "sonly":
                do_hop(1, True, gather_en=False)
                dbg_out(hT[1].ap()[:, 0:1])
            elif _V2T == "hopx":
                do_hop(1, True)
                nc.vector.tensor_tensor(out=gT.ap()[:, :], in0=hT[1].ap()[:, :],
                                        in1=dinvT.ap()[:, :],
                                        op=mybir.AluOpType.mult)
                exchange(gT.ap()[:, :])
                do_hop(2, False)
                dbg_out(hT[2].ap()[:, 0:1])
            elif _V2T == "wp0":
                do_hop(1, True)
                wphase(0)
                dbg_out(hT[0].ap()[:, 0:1])
            for _rep in range(repeat if _V2T == "full" else 0):
                for layer in range(2):
                    first = layer == 0
                    for k in range(1, K + 1):
                        do_hop(k, first and k == 1)
                        if k < K:
                            gcols = gT.ap()[:, :]
                            nc.vector.tensor_tensor(
                                out=gcols, in0=hT[k].ap()[:, :],
                                in1=dinvT.ap()[:, :], op=mybir.AluOpType.mult)
                            exchange(gcols)
                    wphase(layer)
                    if layer == 0:
                        nc.vector.tensor_tensor(
                            out=gT.ap()[:, :], in0=hT[0].ap()[:, :],
                            in1=dinvT.ap()[:, :], op=mybir.AluOpType.mult)
                        exchange(gT.ap()[:, :])

                # pooling: h1 = hT[0]
                Bnm = msgT.ap()[:, 0 : NBLK * 128].rearrange(
                    "p (a b) -> p a b", b=128)
                nc.sync.dma_start(out=msgT.ap()[:, 0 : NBLK * 128],
                                  in_=Bnm_e.ap().bitcast(bf16))
                h1nm = msgT.ap()[:, NBLK * 128 : NBLK * 192].rearrange(
                    "p (a b) -> p a b", b=64)
                nc.sync.dma_start_transpose(h1nm, hT[0].ap()[:, :])
                PT = pp.tile([64, NGR], f32)
                for b in range(NBLK):
                    nc.tensor.matmul(PT[:], lhsT=h1nm[:, b, :], rhs=Bnm[:, b, :],
                                     start=(b == 0), stop=(b == NBLK - 1))
                nc.vector.tensor_copy(out=PTsb[:], in_=PT[:])
                zps = pp.tile([1, NGR], f32, tag="zps")
                nc.tensor.matmul(zps[:], lhsT=Woutb[:], rhs=PTsb[:],
                                 start=True, stop=True)
                zsb = wp.tile([1, NGR], f32, tag="zsb")
                nc.vector.tensor_copy(out=zsb[:], in_=zps[:])
                nc.sync.dma_start(out=ar_in[:], in_=zsb[:])
                nc.gpsimd.collective_compute(
                    "AllReduce", mybir.AluOpType.add,
                    replica_groups=RG, ins=[ar_in[:]], outs=[ar_out[:]],
                )
                res = wp.tile([1, NGR], f32, tag="res")
                nc.sync.dma_start(out=res[:], in_=ar_out[:])
                nc.vector.tensor_scalar_add(res[:], res[:], float(bout))
                nc.sync.dma_start(out=out_e.ap().rearrange("g one -> one g"),
                                  in_=res[:])

    nc.compile()
    return nc, per_core


def kernel(**inputs):
    from concourse.bass_utils import run_bass_kernel_spmd

    nc, per_core = _build(inputs, repeat=1)
    results = run_bass_kernel_spmd(nc, per_core, list(range(8)))
    return results.results[0]["out"].astype(np.float32)


def estimate_hw_time_ns(inputs, r_hi=3, n_rep=8):
    import time
    from concourse.bass_utils import run_bass_kernel_spmd

    walls = {}
    for r in (1, r_hi):
        nc, per_core = _build(inputs, repeat=r)
        run_bass_kernel_spmd(nc, per_core, list(range(8)))  # warm
        ws = []
        for _ in range(n_rep):
            t0 = time.time()
            run_bass_kernel_spmd(nc, per_core, list(range(8)))
            ws.append(time.time() - t0)
        walls[r] = min(ws)
    return (walls[r_hi] - walls[1]) / (r_hi - 1) * 1e9


if __name__ == "__main__":
    import jax
    import reference

    cpu = jax.devices("cpu")[0]
    with jax.default_device(cpu):
        ins = {k: np.asarray(v) for k, v in reference.setup_inputs().items()}
        exp = np.asarray(reference.reference(**ins))
    got = kernel(**ins)
    err = np.abs(got - exp).max() / (np.abs(exp).max() + 1e-12)
    print("rel err:", err)
